# revision 41
# baseline (speedup 1.0000x reference)
"""MoE (8 experts, top-2, sigmoid router, SwiGLU + shared expert) on 8 TRN2 cores.

Strategy: expert-parallel with host-side dispatch. The router (sigmoid scores,
top-2, combine weights) runs on the host in fp32 numpy — verified to match the
jax reference bit-for-bit on expert selection (min 2nd-vs-3rd score gap 1.3e-4
vs ~1e-6 matmul noise). Tokens are gathered per expert, pre-scaled by their
combine weight (silu(s*g)*(s*u) == silu(W1(s*x))*(W3(s*x))), padded to a fixed
capacity C, and dispatched: core e runs a dense SwiGLU for expert e over its
<=C tokens plus the shared expert over a 256-token shard. This cuts device
FLOPs 2.8x vs dense all-experts (top-2 of 8 + shared). Activations are
computed directly in [hidden, token] layout so the down-projection needs no
transposes; weights are pre-tiled on host so every DMA is a single
contiguous >=2KB-per-partition transfer. The host scatter-adds the two expert
contributions per token and adds the shared output.
"""
import numpy as np
import ml_dtypes

import concourse.bass as bass
import concourse.tile as tile
from concourse import bacc, mybir
from concourse.bass_utils import run_bass_kernel_spmd

P = 128
N_CORES = 8
SLEN = 2048
DIM = 2048
HID = 1024
E = 8
TOP_K = 2
SSH = SLEN // N_CORES          # shared-expert tokens per core
DC = DIM // P                  # 16 contraction chunks over dim
HC = HID // P                  # 8 chunks over hidden
TCW = 512                      # max token chunk width (one fp32 PSUM bank)
BF16 = mybir.dt.bfloat16
F32 = mybir.dt.float32
DEF_C = 546                    # routed-token capacity per expert (max is 545)

_CACHE: dict = {}


def _chunks(T):
    # balanced chunks <= TCW (avoids tiny SEQ-bound tail matmuls)
    n = -(-T // TCW)
    base, rem = divmod(T, n)
    out, t0 = [], 0
    for i in range(n):
        w = base + (1 if i < rem else 0)
        out.append((t0, w))
        t0 += w
    return out


def _xch(T):
    # x-chunk widths per branch: small chunks so the very first matmul chain
    # needs only ~0.4MB of x in SBUF (T divisible by 6 resp. 128)
    return [128] * (T // 128) if T <= 256 else [T // 6] * 6


def _build(C):
    nc = bacc.Bacc("TRN2", target_bir_lowering=False, debug=False,
                   num_devices=N_CORES)

    # x layouts: chunk-major [ci, p, dc, t] with dim = dc*128 + p, so each
    # chunk is one contiguous full-rate DMA and the first chain starts early
    xcw_r, xcw_s = _xch(C), _xch(SSH)
    NWARM = 64                 # dummy PE warmup matmuls, sized to end right
    #                            when the first real operands land (~6.4us)
    xr_d = nc.dram_tensor("xr", [len(xcw_r), P, DC, xcw_r[0]], BF16,
                          kind="ExternalInput").ap()
    xs_d = nc.dram_tensor("xs", [len(xcw_s), P, DC, xcw_s[0]], BF16,
                          kind="ExternalInput").ap()
    # fused up+gate weights [ht, p, g/u, dc, h]: lhsT chunks [128 dim, 128 hid]
    wgu_d = nc.dram_tensor("wgu", [HC, P, 2, DC, P], BF16,
                           kind="ExternalInput").ap()
    swgu_d = nc.dram_tensor("swgu", [HC, P, 2, DC, P], BF16,
                            kind="ExternalInput").ap()
    # down weights, 2 dim-tiles per batch [db, p, j, hc, d]
    wd_d = nc.dram_tensor("wd", [DC // 2, P, 2, HC, P], BF16,
                          kind="ExternalInput").ap()
    swd_d = nc.dram_tensor("swd", [DC // 2, P, 2, HC, P], BF16,
                           kind="ExternalInput").ap()
    # outputs [db, d, j, t] with dim = (2*db + j)*128 + d
    yr_d = nc.dram_tensor("yr", [DC // 2, P, 2, C], F32,
                          kind="ExternalOutput").ap()
    ys_d = nc.dram_tensor("ys", [DC // 2, P, 2, SSH], F32,
                          kind="ExternalOutput").ap()

    # routed branch first: its DMA demand rate (~180GB/s) is half the
    # shared branch's early demand, so the load stream never falls behind.
    branches = [
        (C, xcw_r, xr_d, wgu_d, wd_d, yr_d),
        (SSH, xcw_s, xs_d, swgu_d, swd_d, ys_d),
    ]

    # One DMA costs ~630ns on the single shared HWDGE descriptor engine, so
    # DMA count is precious: weight loads are batched (1 per hid-tile / per 2
    # dim-tiles) and issued on the SP queue in an explicit prefetch order;
    # y stores go through the idle Pool engine's SWDGE path, off HWDGE.
    with tile.TileContext(nc) as tc:
        with tc.tile_pool(name="xpool", bufs=1) as xpool, \
             tc.tile_pool(name="hpool", bufs=1) as hpool, \
             tc.tile_pool(name="wpool", bufs=8) as wpool, \
             tc.tile_pool(name="wdpool", bufs=8) as wdpool, \
             tc.tile_pool(name="upsum", bufs=2, space="PSUM") as upsum, \
             tc.tile_pool(name="dpsum", bufs=3, space="PSUM") as dpsum, \
             tc.tile_pool(name="wpsum", bufs=1, space="PSUM") as wpsum, \
             tc.tile_pool(name="tmp", bufs=3) as tmp, \
             tc.tile_pool(name="ypool", bufs=DC // 2) as ypool:

            # ---- explicit load schedule (SP queue order == service order on
            # the single FIFO DMA_ENGINES device). Interleaved so every tile
            # lands just before the PE needs it; see timings in comments.
            xt, wgu_t, wd_t = {}, {}, {}
            for bi, (T, xcw, x_d, gu_d, d_d, y_d) in enumerate(branches):
                xt[bi] = [xpool.tile([P, DC, cw], BF16, tag=f"x{bi}c{ci}",
                                     name="xc")
                          for ci, cw in enumerate(xcw)]

            def _wgu_load(bi, ht, split=False):
                gu_d = branches[bi][3]
                w = wpool.tile([P, 2, DC, P], BF16, tag="wgu", name="wgu")
                if split:          # g half first so the PE can start sooner
                    nc.sync.dma_start(w[:, 0], gu_d[ht][:, 0])
                else:
                    nc.sync.dma_start(w[:], gu_d[ht])
                wgu_t[(bi, ht)] = w
                return w

            def _wd_load(bi, db):
                d_d = branches[bi][4]
                w = wdpool.tile([P, 2, HC, P], BF16, tag="wd", name="wd")
                nc.sync.dma_start(w[:], d_d[db])
                wd_t[(bi, db)] = w

            # PE warmup: dummy matmuls keep the PE busy through the initial
            # DMA wait so the p-state is fully ramped (2.4GHz needs 3us of
            # continuous busy) when the first real chain starts.
            scr = xpool.tile([P, P], BF16, tag="scr", name="scr")
            nc.vector.memset(scr[:], 0.0)
            spsum = wpsum.tile([P, P], F32, tag="warm", name="warm")
            for _ in range(NWARM):
                nc.tensor.matmul(spsum[:], scr[:], scr[:],
                                 start=True, stop=True)

            w0 = _wgu_load(0, 0, split=True)               # g half: ~0.7us
            nc.sync.dma_start(xt[0][0][:], xr_d[0])        # xr chunk0: ~1us
            nc.sync.dma_start(w0[:, 1], wgu_d[0][:, 1])    # u half
            for ci in range(1, len(xcw_r)):
                nc.sync.dma_start(xt[0][ci][:], xr_d[ci])
            w1 = _wgu_load(0, 1, split=True)         # g first: ht1 deadline
            nc.sync.dma_start(w1[:, 1], wgu_d[1][:, 1])
            for ht in range(2, HC):
                _wgu_load(0, ht)                     # done ~30us, used to ~65
            for db in range(DC // 2):
                _wd_load(0, db)                      # done ~42, used 65..95
            for ht in range(HC):
                _wgu_load(1, ht)                     # done ~66, used 95..123
            nc.sync.dma_start(xt[1][0][:], xs_d[0])  # xs, ~68us (used ~95)
            nc.sync.dma_start(xt[1][1][:], xs_d[1])
            for db in range(DC // 2):
                _wd_load(1, db)                      # done ~80, used 123..136

            for bi, (T, xcw, x_d, gu_d, d_d, y_d) in enumerate(branches):
                h = hpool.tile([P, HC, T], BF16, tag=f"h{bi}")
                # ---- up/gate: pg/pu[hid, tok] accumulated over dim chunks
                for ht in range(HC):
                    w = wgu_t[(bi, ht)]
                    t0 = 0
                    for ci, cw in enumerate(xcw):
                        xc = xt[bi][ci]
                        pg = upsum.tile([P, TCW], F32, tag="pg")
                        pu = upsum.tile([P, TCW], F32, tag="pu")
                        for dc in range(DC):
                            st, sp = (dc == 0), (dc == DC - 1)
                            nc.tensor.matmul(pg[:, :cw], w[:, 0, dc, :],
                                             xc[:, dc, :], start=st, stop=sp)
                            nc.tensor.matmul(pu[:, :cw], w[:, 1, dc, :],
                                             xc[:, dc, :], start=st, stop=sp)
                        sg = tmp.tile([P, TCW], BF16, tag="sg")
                        su = tmp.tile([P, TCW], BF16, tag="su")
                        nc.scalar.activation(sg[:, :cw], pg[:, :cw],
                                             mybir.ActivationFunctionType.Silu)
                        nc.vector.tensor_copy(su[:, :cw], pu[:, :cw])
                        nc.vector.tensor_mul(h[:, ht, t0:t0 + cw],
                                             sg[:, :cw], su[:, :cw])
                        t0 += cw
                # ---- down: py[dim, tok] accumulated over hidden chunks.
                # y staged in resident SBUF (bufs cover the branch) so store
                # transfer latency never blocks the PE; stores via Pool/SWDGE.
                last = (bi == len(branches) - 1)
                for db in range(DC // 2):
                    w = wd_t[(bi, db)]
                    ys = ypool.tile([P, 2, T], F32, tag=f"yt{bi}",
                                    name="ystage")
                    for j in range(2):
                        tail = last and db == DC // 2 - 1 and j == 1
                        # tiny final window + its store on the idle SP queue
                        # (a dma_start holds its engine's SEQ for the 632ns
                        # HWDGE desc-gen, so keep stores off the copy engine;
                        # earlier tail stores go via Pool/SWDGE so only the
                        # final desc-gen sits on the HWDGE critical path)
                        wins = ([(0, T - 32), (T - 32, 32)]
                                if tail else _chunks(T))
                        for (t0, tw) in wins:
                            py = dpsum.tile([P, TCW], F32, tag="py")
                            for hc in range(HC):
                                nc.tensor.matmul(py[:, :tw], w[:, j, hc, :],
                                                 h[:, hc, t0:t0 + tw],
                                                 start=(hc == 0),
                                                 stop=(hc == HC - 1))
                            nc.scalar.copy(ys[:, j, t0:t0 + tw], py[:, :tw])
                            if tail:
                                eng = nc.scalar if tw == 32 else nc.gpsimd
                                eng.dma_start(y_d[db, :, j, t0:t0 + tw],
                                              ys[:, j, t0:t0 + tw])
                        if last and db == DC // 2 - 1 and j == 0:
                            nc.sync.dma_start(y_d[db, :, 0, :], ys[:, 0, :])
                    if not (last and db == DC // 2 - 1):
                        nc.gpsimd.dma_start(y_d[db], ys[:])

    nc.compile()
    return nc


def _get_nc(C=None):
    if C is None:
        C = _CACHE.get("last_C", DEF_C)
    if ("nc", C) not in _CACHE:
        _CACHE[("nc", C)] = _build(C)
    _CACHE["last_C"] = C
    return _CACHE[("nc", C)]


def _bf16(a):
    return np.ascontiguousarray(a.astype(ml_dtypes.bfloat16))


def _wgu_layout(wg, wu):
    # wg/wu: [HID, DIM] -> fused [ht, p, g/u, dc, h]
    g = wg.reshape(HC, P, DC, P).transpose(0, 3, 2, 1)
    u = wu.reshape(HC, P, DC, P).transpose(0, 3, 2, 1)
    return np.ascontiguousarray(np.stack([g, u], axis=2))


def _wd_layout(w):
    # w: [DIM, HID] -> [db, p, j, hc, d] (2 dim-tiles per batch)
    return np.ascontiguousarray(
        w.reshape(DC // 2, 2, P, HC, P).transpose(0, 4, 1, 3, 2))


def _x_layout(rows, T):
    # rows: [n, DIM] bf16 -> chunk-major [ci, p, dc, t] padded to T tokens
    arr = np.zeros((T, DIM), dtype=ml_dtypes.bfloat16)
    arr[:rows.shape[0]] = rows
    cw = _xch(T)[0]
    return np.ascontiguousarray(
        arr.reshape(T // cw, cw, DC, P).transpose(0, 3, 2, 1))


def kernel(x, gate, expert_bias, w1, w2, w3, sw1, sw2, sw3, _want_results=False):
    x = np.asarray(x, dtype=np.float32)
    gate = np.ascontiguousarray(np.asarray(gate, dtype=np.float32))
    expert_bias = np.asarray(expert_bias, dtype=np.float32)
    w1 = np.asarray(w1, dtype=np.float32)
    w2 = np.asarray(w2, dtype=np.float32)
    w3 = np.asarray(w3, dtype=np.float32)

    xt = x.reshape(SLEN, DIM)

    # ---- host router (fp32, matches jax top-2 selection on this regime)
    logits = xt @ gate
    scores = 1.0 / (1.0 + np.exp(-logits))
    v = scores + expert_bias[None, :]
    top2 = np.argpartition(-v, TOP_K - 1, axis=1)[:, :TOP_K]      # unordered
    s_top = np.take_along_axis(scores, top2, axis=1)

    e_flat = top2.ravel()
    tok_flat = np.repeat(np.arange(SLEN), TOP_K)
    s_flat = s_top.ravel()
    order = np.argsort(e_flat, kind="stable")
    counts = np.bincount(e_flat, minlength=E)
    offs = np.concatenate([[0], np.cumsum(counts)])

    C = max(DEF_C, int(-(-counts.max() // 6) * 6))  # DEF_C covers max 545

    # pre-scaled routed tokens, expert-sorted
    xs_rows = (xt[tok_flat[order]] * s_flat[order][:, None]).astype(
        ml_dtypes.bfloat16)

    # ---- per-core inputs
    wgu_all = [_wgu_layout(_bf16(w1[e]), _bf16(w3[e])) for e in range(E)]
    wd_all = [_wd_layout(_bf16(w2[e])) for e in range(E)]
    swgu = _wgu_layout(_bf16(np.asarray(sw1, np.float32)),
                       _bf16(np.asarray(sw3, np.float32)))
    swd = _wd_layout(_bf16(np.asarray(sw2, np.float32)))

    in_maps = []
    for c in range(N_CORES):
        rows = xs_rows[offs[c]:offs[c + 1]]
        xr = _x_layout(rows, C)
        xsh = _x_layout(_bf16(xt[c * SSH:(c + 1) * SSH]), SSH)
        in_maps.append({
            "xr": xr, "xs": xsh,
            "wgu": wgu_all[c], "wd": wd_all[c],
            "swgu": swgu, "swd": swd,
        })

    nc = _get_nc(C)
    res = run_bass_kernel_spmd(nc, in_maps, list(range(N_CORES)))

    # ---- host combine
    routed_rows = np.empty((SLEN * TOP_K, DIM), dtype=np.float32)
    shared = np.empty((SLEN, DIM), dtype=np.float32)
    for c in range(N_CORES):
        yr = res.results[c]["yr"]                   # [DC/2, P, 2, C]
        ys = res.results[c]["ys"]                   # [DC/2, P, 2, SSH]
        n = counts[c]
        routed_rows[order[offs[c]:offs[c + 1]]] = \
            yr.transpose(3, 0, 2, 1).reshape(C, DIM)[:n]
        shared[c * SSH:(c + 1) * SSH] = \
            ys.transpose(3, 0, 2, 1).reshape(SSH, DIM)

    routed = routed_rows.reshape(SLEN, TOP_K, DIM).sum(axis=1)
    out = (routed + shared).reshape(1, 1, SLEN, DIM).astype(np.float32)
    if _want_results:
        return out, res
    return out


# revision 42
# speedup vs baseline: 1.0007x; 1.0007x over previous
"""MoE (8 experts, top-2, sigmoid router, SwiGLU + shared expert) on 8 TRN2 cores.

Strategy: expert-parallel with host-side dispatch. The router (sigmoid scores,
top-2, combine weights) runs on the host in fp32 numpy — verified to match the
jax reference bit-for-bit on expert selection (min 2nd-vs-3rd score gap 1.3e-4
vs ~1e-6 matmul noise). Tokens are gathered per expert, pre-scaled by their
combine weight (silu(s*g)*(s*u) == silu(W1(s*x))*(W3(s*x))), padded to a fixed
capacity C, and dispatched: core e runs a dense SwiGLU for expert e over its
<=C tokens plus the shared expert over a 256-token shard. This cuts device
FLOPs 2.8x vs dense all-experts (top-2 of 8 + shared). Activations are
computed directly in [hidden, token] layout so the down-projection needs no
transposes; weights are pre-tiled on host so every DMA is a single
contiguous >=2KB-per-partition transfer. The host scatter-adds the two expert
contributions per token and adds the shared output.
"""
import numpy as np
import ml_dtypes

import concourse.bass as bass
import concourse.tile as tile
from concourse import bacc, mybir
from concourse.bass_utils import run_bass_kernel_spmd

P = 128
N_CORES = 8
SLEN = 2048
DIM = 2048
HID = 1024
E = 8
TOP_K = 2
SSH = SLEN // N_CORES          # shared-expert tokens per core
DC = DIM // P                  # 16 contraction chunks over dim
HC = HID // P                  # 8 chunks over hidden
TCW = 512                      # max token chunk width (one fp32 PSUM bank)
BF16 = mybir.dt.bfloat16
F32 = mybir.dt.float32
DEF_C = 546                    # routed-token capacity per expert (max is 545)

_CACHE: dict = {}


def _chunks(T):
    # balanced chunks <= TCW (avoids tiny SEQ-bound tail matmuls)
    n = -(-T // TCW)
    base, rem = divmod(T, n)
    out, t0 = [], 0
    for i in range(n):
        w = base + (1 if i < rem else 0)
        out.append((t0, w))
        t0 += w
    return out


def _xch(T):
    # x-chunk widths per branch: small chunks so the very first matmul chain
    # needs only ~0.4MB of x in SBUF (T divisible by 6 resp. 128)
    return [128] * (T // 128) if T <= 256 else [T // 6] * 6


def _build(C):
    nc = bacc.Bacc("TRN2", target_bir_lowering=False, debug=False,
                   num_devices=N_CORES)

    # x layouts: chunk-major [ci, p, dc, t] with dim = dc*128 + p, so each
    # chunk is one contiguous full-rate DMA and the first chain starts early
    xcw_r, xcw_s = _xch(C), _xch(SSH)
    NWARM = 64                 # dummy PE warmup matmuls, sized to end right
    #                            when the first real operands land (~6.4us)
    xr_d = nc.dram_tensor("xr", [len(xcw_r), P, DC, xcw_r[0]], BF16,
                          kind="ExternalInput").ap()
    xs_d = nc.dram_tensor("xs", [len(xcw_s), P, DC, xcw_s[0]], BF16,
                          kind="ExternalInput").ap()
    # fused up+gate weights [ht, p, g/u, dc, h]: lhsT chunks [128 dim, 128 hid]
    wgu_d = nc.dram_tensor("wgu", [HC, P, 2, DC, P], BF16,
                           kind="ExternalInput").ap()
    swgu_d = nc.dram_tensor("swgu", [HC, P, 2, DC, P], BF16,
                            kind="ExternalInput").ap()
    # down weights, 2 dim-tiles per batch [db, p, j, hc, d]
    wd_d = nc.dram_tensor("wd", [DC // 2, P, 2, HC, P], BF16,
                          kind="ExternalInput").ap()
    swd_d = nc.dram_tensor("swd", [DC // 2, P, 2, HC, P], BF16,
                           kind="ExternalInput").ap()
    # outputs [db, d, j, t] with dim = (2*db + j)*128 + d
    yr_d = nc.dram_tensor("yr", [DC // 2, P, 2, C], F32,
                          kind="ExternalOutput").ap()
    ys_d = nc.dram_tensor("ys", [DC // 2, P, 2, SSH], F32,
                          kind="ExternalOutput").ap()

    # routed branch first: its DMA demand rate (~180GB/s) is half the
    # shared branch's early demand, so the load stream never falls behind.
    branches = [
        (C, xcw_r, xr_d, wgu_d, wd_d, yr_d),
        (SSH, xcw_s, xs_d, swgu_d, swd_d, ys_d),
    ]

    # One DMA costs ~630ns on the single shared HWDGE descriptor engine, so
    # DMA count is precious: weight loads are batched (1 per hid-tile / per 2
    # dim-tiles) and issued on the SP queue in an explicit prefetch order;
    # y stores go through the idle Pool engine's SWDGE path, off HWDGE.
    with tile.TileContext(nc) as tc:
        with tc.tile_pool(name="xpool", bufs=1) as xpool, \
             tc.tile_pool(name="hpool", bufs=1) as hpool, \
             tc.tile_pool(name="wpool", bufs=8) as wpool, \
             tc.tile_pool(name="wdpool", bufs=8) as wdpool, \
             tc.tile_pool(name="upsum", bufs=2, space="PSUM") as upsum, \
             tc.tile_pool(name="dpsum", bufs=3, space="PSUM") as dpsum, \
             tc.tile_pool(name="wpsum", bufs=1, space="PSUM") as wpsum, \
             tc.tile_pool(name="tmp", bufs=3) as tmp, \
             tc.tile_pool(name="ypool", bufs=DC // 2) as ypool:

            # ---- explicit load schedule (SP queue order == service order on
            # the single FIFO DMA_ENGINES device). Interleaved so every tile
            # lands just before the PE needs it; see timings in comments.
            xt, wgu_t, wd_t = {}, {}, {}
            for bi, (T, xcw, x_d, gu_d, d_d, y_d) in enumerate(branches):
                xt[bi] = [xpool.tile([P, DC, cw], BF16, tag=f"x{bi}c{ci}",
                                     name="xc")
                          for ci, cw in enumerate(xcw)]

            def _wgu_load(bi, ht, split=False):
                gu_d = branches[bi][3]
                w = wpool.tile([P, 2, DC, P], BF16, tag="wgu", name="wgu")
                if split:          # g half first so the PE can start sooner
                    nc.sync.dma_start(w[:, 0], gu_d[ht][:, 0])
                else:
                    nc.sync.dma_start(w[:], gu_d[ht])
                wgu_t[(bi, ht)] = w
                return w

            def _wd_load(bi, db):
                d_d = branches[bi][4]
                w = wdpool.tile([P, 2, HC, P], BF16, tag="wd", name="wd")
                nc.sync.dma_start(w[:], d_d[db])
                wd_t[(bi, db)] = w

            # PE warmup: dummy matmuls keep the PE busy through the initial
            # DMA wait so the p-state is fully ramped (2.4GHz needs 3us of
            # continuous busy) when the first real chain starts.
            scr = xpool.tile([P, P], BF16, tag="scr", name="scr")
            nc.vector.memset(scr[:], 0.0)
            spsum = wpsum.tile([P, P], F32, tag="warm", name="warm")
            for _ in range(NWARM):
                nc.tensor.matmul(spsum[:], scr[:], scr[:],
                                 start=True, stop=True)

            w0 = _wgu_load(0, 0, split=True)               # g half: ~0.7us
            nc.sync.dma_start(xt[0][0][:], xr_d[0])        # xr chunk0: ~1us
            nc.sync.dma_start(w0[:, 1], wgu_d[0][:, 1])    # u half
            for ci in range(1, len(xcw_r)):
                nc.sync.dma_start(xt[0][ci][:], xr_d[ci])
            w1 = _wgu_load(0, 1, split=True)         # g first: ht1 deadline
            nc.sync.dma_start(w1[:, 1], wgu_d[1][:, 1])
            for ht in range(2, HC):
                _wgu_load(0, ht)                     # done ~30us, used to ~65
            for db in range(DC // 2):
                _wd_load(0, db)                      # done ~42, used 65..95
            for ht in range(HC):
                _wgu_load(1, ht)                     # done ~66, used 95..123
            nc.sync.dma_start(xt[1][0][:], xs_d[0])  # xs, ~68us (used ~95)
            nc.sync.dma_start(xt[1][1][:], xs_d[1])
            for db in range(DC // 2):
                _wd_load(1, db)                      # done ~80, used 123..136

            for bi, (T, xcw, x_d, gu_d, d_d, y_d) in enumerate(branches):
                h = hpool.tile([P, HC, T], BF16, tag=f"h{bi}")
                # ---- up/gate: pg/pu[hid, tok] accumulated over dim chunks
                for ht in range(HC):
                    w = wgu_t[(bi, ht)]
                    t0 = 0
                    for ci, cw in enumerate(xcw):
                        xc = xt[bi][ci]
                        pg = upsum.tile([P, TCW], F32, tag="pg")
                        pu = upsum.tile([P, TCW], F32, tag="pu")
                        for dc in range(DC):
                            st, sp = (dc == 0), (dc == DC - 1)
                            nc.tensor.matmul(pg[:, :cw], w[:, 0, dc, :],
                                             xc[:, dc, :], start=st, stop=sp)
                            nc.tensor.matmul(pu[:, :cw], w[:, 1, dc, :],
                                             xc[:, dc, :], start=st, stop=sp)
                        sg = tmp.tile([P, TCW], BF16, tag="sg")
                        su = tmp.tile([P, TCW], BF16, tag="su")
                        nc.scalar.activation(sg[:, :cw], pg[:, :cw],
                                             mybir.ActivationFunctionType.Silu)
                        nc.vector.tensor_copy(su[:, :cw], pu[:, :cw])
                        nc.vector.tensor_mul(h[:, ht, t0:t0 + cw],
                                             sg[:, :cw], su[:, :cw])
                        t0 += cw
                # ---- down: py[dim, tok] accumulated over hidden chunks.
                # y staged in resident SBUF (bufs cover the branch) so store
                # transfer latency never blocks the PE; stores via Pool/SWDGE.
                last = (bi == len(branches) - 1)
                for db in range(DC // 2):
                    w = wd_t[(bi, db)]
                    ys = ypool.tile([P, 2, T], F32, tag=f"yt{bi}",
                                    name="ystage")
                    for j in range(2):
                        tail = last and db == DC // 2 - 1 and j == 1
                        # tiny final window + its store on the idle SP queue
                        # (a dma_start holds its engine's SEQ for the 632ns
                        # HWDGE desc-gen, so keep stores off the copy engine;
                        # earlier tail stores go via Pool/SWDGE so only the
                        # final desc-gen sits on the HWDGE critical path)
                        wins = ([(0, T - 64), (T - 64, 64)]
                                if tail else _chunks(T))
                        for (t0, tw) in wins:
                            py = dpsum.tile([P, TCW], F32, tag="py")
                            for hc in range(HC):
                                nc.tensor.matmul(py[:, :tw], w[:, j, hc, :],
                                                 h[:, hc, t0:t0 + tw],
                                                 start=(hc == 0),
                                                 stop=(hc == HC - 1))
                            nc.scalar.copy(ys[:, j, t0:t0 + tw], py[:, :tw])
                            if tail:
                                eng = nc.scalar if tw == 64 else nc.sync
                                eng.dma_start(y_d[db, :, j, t0:t0 + tw],
                                              ys[:, j, t0:t0 + tw])
                        if last and db == DC // 2 - 1 and j == 0:
                            nc.sync.dma_start(y_d[db, :, 0, :], ys[:, 0, :])
                    if not (last and db == DC // 2 - 1):
                        nc.gpsimd.dma_start(y_d[db], ys[:])

    nc.compile()
    return nc


def _get_nc(C=None):
    if C is None:
        C = _CACHE.get("last_C", DEF_C)
    if ("nc", C) not in _CACHE:
        _CACHE[("nc", C)] = _build(C)
    _CACHE["last_C"] = C
    return _CACHE[("nc", C)]


def _bf16(a):
    return np.ascontiguousarray(a.astype(ml_dtypes.bfloat16))


def _wgu_layout(wg, wu):
    # wg/wu: [HID, DIM] -> fused [ht, p, g/u, dc, h]
    g = wg.reshape(HC, P, DC, P).transpose(0, 3, 2, 1)
    u = wu.reshape(HC, P, DC, P).transpose(0, 3, 2, 1)
    return np.ascontiguousarray(np.stack([g, u], axis=2))


def _wd_layout(w):
    # w: [DIM, HID] -> [db, p, j, hc, d] (2 dim-tiles per batch)
    return np.ascontiguousarray(
        w.reshape(DC // 2, 2, P, HC, P).transpose(0, 4, 1, 3, 2))


def _x_layout(rows, T):
    # rows: [n, DIM] bf16 -> chunk-major [ci, p, dc, t] padded to T tokens
    arr = np.zeros((T, DIM), dtype=ml_dtypes.bfloat16)
    arr[:rows.shape[0]] = rows
    cw = _xch(T)[0]
    return np.ascontiguousarray(
        arr.reshape(T // cw, cw, DC, P).transpose(0, 3, 2, 1))


def kernel(x, gate, expert_bias, w1, w2, w3, sw1, sw2, sw3, _want_results=False):
    x = np.asarray(x, dtype=np.float32)
    gate = np.ascontiguousarray(np.asarray(gate, dtype=np.float32))
    expert_bias = np.asarray(expert_bias, dtype=np.float32)
    w1 = np.asarray(w1, dtype=np.float32)
    w2 = np.asarray(w2, dtype=np.float32)
    w3 = np.asarray(w3, dtype=np.float32)

    xt = x.reshape(SLEN, DIM)

    # ---- host router (fp32, matches jax top-2 selection on this regime)
    logits = xt @ gate
    scores = 1.0 / (1.0 + np.exp(-logits))
    v = scores + expert_bias[None, :]
    top2 = np.argpartition(-v, TOP_K - 1, axis=1)[:, :TOP_K]      # unordered
    s_top = np.take_along_axis(scores, top2, axis=1)

    e_flat = top2.ravel()
    tok_flat = np.repeat(np.arange(SLEN), TOP_K)
    s_flat = s_top.ravel()
    order = np.argsort(e_flat, kind="stable")
    counts = np.bincount(e_flat, minlength=E)
    offs = np.concatenate([[0], np.cumsum(counts)])

    C = max(DEF_C, int(-(-counts.max() // 6) * 6))  # DEF_C covers max 545

    # pre-scaled routed tokens, expert-sorted
    xs_rows = (xt[tok_flat[order]] * s_flat[order][:, None]).astype(
        ml_dtypes.bfloat16)

    # ---- per-core inputs
    wgu_all = [_wgu_layout(_bf16(w1[e]), _bf16(w3[e])) for e in range(E)]
    wd_all = [_wd_layout(_bf16(w2[e])) for e in range(E)]
    swgu = _wgu_layout(_bf16(np.asarray(sw1, np.float32)),
                       _bf16(np.asarray(sw3, np.float32)))
    swd = _wd_layout(_bf16(np.asarray(sw2, np.float32)))

    in_maps = []
    for c in range(N_CORES):
        rows = xs_rows[offs[c]:offs[c + 1]]
        xr = _x_layout(rows, C)
        xsh = _x_layout(_bf16(xt[c * SSH:(c + 1) * SSH]), SSH)
        in_maps.append({
            "xr": xr, "xs": xsh,
            "wgu": wgu_all[c], "wd": wd_all[c],
            "swgu": swgu, "swd": swd,
        })

    nc = _get_nc(C)
    res = run_bass_kernel_spmd(nc, in_maps, list(range(N_CORES)))

    # ---- host combine
    routed_rows = np.empty((SLEN * TOP_K, DIM), dtype=np.float32)
    shared = np.empty((SLEN, DIM), dtype=np.float32)
    for c in range(N_CORES):
        yr = res.results[c]["yr"]                   # [DC/2, P, 2, C]
        ys = res.results[c]["ys"]                   # [DC/2, P, 2, SSH]
        n = counts[c]
        routed_rows[order[offs[c]:offs[c + 1]]] = \
            yr.transpose(3, 0, 2, 1).reshape(C, DIM)[:n]
        shared[c * SSH:(c + 1) * SSH] = \
            ys.transpose(3, 0, 2, 1).reshape(SSH, DIM)

    routed = routed_rows.reshape(SLEN, TOP_K, DIM).sum(axis=1)
    out = (routed + shared).reshape(1, 1, SLEN, DIM).astype(np.float32)
    if _want_results:
        return out, res
    return out
